# revision 11
# baseline (speedup 1.0000x reference)
"""Causal self-attention (B=2, T=2048, C=1024, H=16, D=64) on 8 TRN2 NeuronCores.

Tensor-parallel over heads: each core owns 2 heads. w_qkv columns and w_out
rows are sharded by head; x (transposed on host) is replicated. Each core
computes qkv projection -> causal attention -> partial output projection for
its heads; the host sums the 8 partials (the TP all-reduce) and adds b_out.

Device layouts (per core):
  xT      [C=1024, BT=4096]   x transposed (fp16), replicated
  wqkv    [1024, 384]         w_qkv cols  [q_h0 q_h1 k_h0 k_h1 v_h0 v_h1]*64
  bqkv    [128, 3]            matching bias columns per m-tile (fp32)
  wout    [128, 1024]         w_out rows  [h0 h1]*64
  outp    [4096, 1024]        partial output (pre-b_out, fp16)

Matmul operands are fp16 (1 cycle/row, 2-byte weight loads, PSUM accumulates
fp32); scores/softmax stats stay fp32. Scores are computed transposed (keys
on partitions) so softmax needs no transposes: denominators come free from a
ones-column appended to V, and the causal mask is an affine_select on the
exp'd tile. Diagonal score tiles only compute/exp the valid column range
(query >= key); fully masked work is skipped.

The PE sustains only partial utilization once the power throttle engages, so
the schedule keeps the PE queue free of waits: the score->exp->PV chain is
software-pipelined with a global DEPTH-item lag between the score matmul
(stage 1: matmul + exp [+ causal select]) and the PV matmul (stage 2),
interleaving both heads. Per-head normalization is a three-hop chain
(scalar-engine reciprocal straight off the PSUM denominator row ->
partition_broadcast -> in-place multiply); it and the output projection
(+ fp16 partial store) ride a second queue lagged by stage-1 emission count
so their PE work lands only after the chain has had time to complete, and
stores overlap attention compute instead of forming a serial tail.
"""

from collections import deque

import numpy as np

import concourse.bass as bass
from concourse import bacc
import concourse.bass_utils as bass_utils
import concourse.mybir as mybir
from concourse.masks import make_identity
from concourse.tile import TileContext

B, T, C, H, D = 2, 2048, 1024, 16, 64
BT = B * T
NCORES = 8
HPC = H // NCORES          # heads per core
JL = 3 * HPC * D           # 384 local qkv output columns
CL = HPC * D               # 128 local channels into out-proj
KT = 128                   # keys per tile (partition dim of scores^T)
QB = 512                   # queries per block (free dim of scores^T)
NQB = T // QB
NKT = QB // KT             # key tiles per query block (diagonal tiles)
DEPTH = 3                  # stage1 -> stage2 software-pipeline distance
LAG_N = 2                  # stage1 emissions before a head's norm chain
LAG_O = 10                 # stage1 emissions before a block's out-proj
F32 = mybir.dt.float32
F16 = mybir.dt.float16
AF = mybir.ActivationFunctionType

_cache = {}


def _build_bass():
    nc = bacc.Bacc("TRN2", target_bir_lowering=False, debug=False)
    xT = nc.dram_tensor("xT", [C, BT], F16, kind="ExternalInput").ap()
    wqkv = nc.dram_tensor("wqkv", [C, JL], F16, kind="ExternalInput").ap()
    bqkv = nc.dram_tensor("bqkv", [128, 3], F32, kind="ExternalInput").ap()
    wout = nc.dram_tensor("wout", [CL, C], F16, kind="ExternalInput").ap()
    outp = nc.dram_tensor("outp", [BT, C], F16, kind="ExternalOutput").ap()

    with TileContext(nc) as tc:
        with (
            tc.tile_pool(name="const", bufs=1) as const,
            tc.tile_pool(name="xtp", bufs=3) as xtp,
            tc.tile_pool(name="vtp", bufs=3) as vtp,
            tc.tile_pool(name="ptp", bufs=6) as ptp,
            tc.tile_pool(name="stg", bufs=3) as stg,
            tc.tile_pool(name="rbp", bufs=3) as rbp,
            tc.tile_pool(name="obp", bufs=4) as obp,
            tc.tile_pool(name="psS", bufs=3, space="PSUM") as psS,
            tc.tile_pool(name="psT", bufs=1, space="PSUM") as psT,
            tc.tile_pool(name="psPV", bufs=2, space="PSUM") as psPV,
            tc.tile_pool(name="psO", bufs=2, space="PSUM") as psO,
        ):
            # ---- static tensors (DMA order tuned: q-weights then the first
            # x tile, so the PE starts ~2us in instead of waiting for the
            # whole weight set; wout is only needed ~90us in)
            wm_sb = []
            wm_sb.append(const.tile([128, 8, 128], F16, name="wq_sb"))
            nc.sync.dma_start(
                out=wm_sb[0],
                in_=wqkv[:, 0:128].rearrange("(k p) j -> p k j", p=128))
            xt0 = xtp.tile([128, 8, QB], F16, tag="xt", name="xt")
            nc.sync.dma_start(
                out=xt0, in_=xT[:, 0:QB].rearrange("(k p) t -> p k t", p=128))
            for m in (1, 2):
                wm = const.tile([128, 8, 128], F16, name=f"wm{m}_sb")
                nc.sync.dma_start(
                    out=wm,
                    in_=wqkv[:, m * 128:(m + 1) * 128].rearrange(
                        "(k p) j -> p k j", p=128))
                wm_sb.append(wm)
            bias_sb = const.tile([128, 3], F32)
            nc.sync.dma_start(out=bias_sb, in_=bqkv)
            ident = const.tile([128, 128], F16)
            make_identity(nc, ident)
            qT = const.tile([128, BT], F16)    # rows: [h0 d64 | h1 d64]
            kTt = const.tile([128, BT], F16)
            # V in [t, d] tiles + ones column for softmax denominators
            v_sb = const.tile([128, HPC, B, T // KT, D + 1], F16)
            for h in range(HPC):
                for b_ in range(B):
                    nc.vector.memset(v_sb[:, h, b_, :, D:D + 1], 1.0)
            attnTc = const.tile([128, BT], F16)
            attnT1 = const.tile([64, BT], F16)

            # ---- phase A: qkv projection (qkv^T layout) + V transpose
            for tb in range(BT // QB):
                if tb == 0:
                    xt = xt0
                else:
                    xt = xtp.tile([128, 8, QB], F16, tag="xt", name="xt")
                    q_ = nc.sync if tb % 2 == 0 else nc.scalar
                    q_.dma_start(
                        out=xt,
                        in_=xT[:, tb * QB:(tb + 1) * QB].rearrange(
                            "(k p) t -> p k t", p=128))
                for m in range(3):
                    ps = psS.tile([128, QB], F32, tag="s", name="psp")
                    for k in range(8):
                        nc.tensor.matmul(
                            ps,
                            lhsT=wm_sb[m][:, k, :],
                            rhs=xt[:, k, :],
                            start=(k == 0), stop=(k == 7))
                    if m == 0:
                        nc.scalar.activation(
                            out=qT[:, tb * QB:(tb + 1) * QB], in_=ps,
                            func=AF.Identity, bias=bias_sb[:, 0:1])
                    elif m == 1:
                        nc.scalar.activation(
                            out=kTt[:, tb * QB:(tb + 1) * QB], in_=ps,
                            func=AF.Identity, bias=bias_sb[:, 1:2])
                    else:
                        vt = vtp.tile([128, QB], F16, tag="vt", name="vt")
                        nc.scalar.activation(
                            out=vt, in_=ps, func=AF.Identity, bias=bias_sb[:, 2:3])
                        for c4 in range(QB // 128):
                            t0 = tb * QB + c4 * 128
                            b_, kt = t0 // T, (t0 % T) // KT
                            for h in range(HPC):
                                pst = psT.tile([128, D], F16, tag="pst", name="pst")
                                nc.tensor.transpose(
                                    pst,
                                    vt[h * 64:(h + 1) * 64, c4 * 128:(c4 + 1) * 128],
                                    ident[h * 64:(h + 1) * 64, h * 64:(h + 1) * 64])
                                nc.vector.tensor_copy(
                                    out=v_sb[:, h, b_, kt, 0:D], in_=pst)
            wout_sb = const.tile([128, C], F16)
            nc.sync.dma_start(out=wout_sb, in_=wout)

            # ---- fused phase B+C: attention, normalization, out-proj
            # Stage 1 (emitted immediately): score matmul + exp (+ causal
            # select on diagonal tiles). Stage 2 (DEPTH stage-1 emissions
            # later, via `pending`): the PV matmul, so the PE never waits on
            # the scalar-engine exp. Norm + out-proj closures ride `pending2`
            # keyed on stage-1 emission count.
            seq = [0]
            pending = deque()
            pending2 = deque()   # (due_seq, closure)
            pv_t = {}

            def stage1(b_, qb, h, kt, diag):
                hs = slice(h * 64, (h + 1) * 64)
                q0 = b_ * T + qb * QB
                off = (kt - NKT * qb) * KT if diag else 0
                w = QB - off
                ps = psS.tile([128, w], F32, tag="s", name="pss")
                nc.tensor.matmul(
                    ps,
                    lhsT=kTt[hs, b_ * T + kt * KT: b_ * T + (kt + 1) * KT],
                    rhs=qT[hs, q0 + off:q0 + QB],
                    start=True, stop=True)
                pt = ptp.tile([128, w], F16, tag="pt", name="pt")
                nc.scalar.activation(
                    out=pt, in_=ps, func=AF.Exp, scale=float(D) ** -0.5)
                if diag:
                    # keep exp(score) where local query col >= key row
                    nc.gpsimd.affine_select(
                        out=pt, in_=pt,
                        compare_op=mybir.AluOpType.is_ge, fill=0.0,
                        base=0, channel_multiplier=-1, pattern=[[1, w]])
                return pt, off

            def norm_head(b_, qb, h, dstage):
                cols = slice(b_ * T + qb * QB, b_ * T + (qb + 1) * QB)
                # den row partition 64 -> 0, reciprocal + fp16 cast there,
                # broadcast (from partition 0, the validated pattern), mul
                d0 = stg.tile([1, QB], F32, tag=f"d0{h}", name="d0")
                nc.sync.dma_start(out=d0, in_=dstage[D:D + 1, :])
                r32 = stg.tile([1, QB], F32, tag=f"r32{h}", name="r32")
                nc.vector.reciprocal(out=r32, in_=d0)
                r16 = stg.tile([1, QB], F16, tag=f"r16{h}", name="r16")
                nc.vector.tensor_copy(out=r16, in_=r32)
                # partition_broadcast honors neither input nor output
                # partition offsets, so rb lives at partitions 0:64 and h1 is
                # normalized in attnT1 (also 0:64) before the partition move.
                rb = rbp.tile([D, QB], F16, tag=f"rb{h}", name="rb")
                nc.gpsimd.partition_broadcast(rb, r16)
                if h == 0:
                    sl = attnTc[0:D, cols]
                    nc.vector.tensor_mul(sl, sl, rb)
                else:
                    sl = attnT1[:, cols]
                    nc.vector.tensor_mul(sl, sl, rb)
                    nc.sync.dma_start(
                        out=attnTc[D:2 * D, cols], in_=attnT1[:, cols])

            def stage2(b_, qb, h, kt, pt, off):
                q0 = b_ * T + qb * QB
                n_kt = (qb + 1) * NKT
                key = (b_, qb, h)
                if key not in pv_t:
                    pv_t[key] = psPV.tile([D + 1, QB], F32, tag="pv", name="pv")
                pv = pv_t[key]
                nc.tensor.matmul(
                    pv[:, off:QB],
                    lhsT=v_sb[:, h, b_, kt, :],
                    rhs=pt,
                    start=(kt == 0), stop=(kt == n_kt - 1))
                if kt == n_kt - 1:
                    if h == 0:
                        nc.vector.tensor_copy(
                            out=attnTc[0:D, q0:q0 + QB], in_=pv[0:D, :])
                    else:
                        nc.vector.tensor_copy(
                            out=attnT1[:, q0:q0 + QB], in_=pv[0:D, :])
                    dstage = stg.tile(
                        [D + 1, QB], F32, tag=f"dst{h}", name="dstage")
                    nc.vector.tensor_copy(
                        out=dstage[D:D + 1, :], in_=pv[D:D + 1, :])
                    pending2.append(
                        (seq[0] + LAG_N,
                         lambda b_=b_, qb=qb, h=h, d=dstage:
                         norm_head(b_, qb, h, d)))
                    if h == HPC - 1:
                        for j in range(QB // 128):
                            t0 = q0 + j * 128
                            for ch in range(2):
                                pending2.append(
                                    (seq[0] + LAG_O,
                                     lambda t0=t0, ch=ch: outproj(t0, ch)))
                    del pv_t[key]

            def outproj(t0, ch):
                po = psO.tile([128, QB], F32, tag="po", name="po")
                nc.tensor.matmul(
                    po,
                    lhsT=attnTc[:, t0:t0 + 128],
                    rhs=wout_sb[:, ch * QB:(ch + 1) * QB],
                    start=True, stop=True)
                ob = obp.tile([128, QB], F16, tag="ob", name="ob")
                nc.any.tensor_copy(out=ob, in_=po)
                nc.sync.dma_start(
                    out=outp[t0:t0 + 128, ch * QB:(ch + 1) * QB], in_=ob)

            for b_ in range(B):
                for qb in range(NQB):
                    blk = []
                    for kt in range(NKT * qb):
                        for h in range(HPC):
                            blk.append((b_, qb, h, kt, False))
                    for kt in range(NKT * qb, NKT * (qb + 1)):
                        for h in range(HPC):
                            blk.append((b_, qb, h, kt, True))
                    for it in blk:
                        pt, off = stage1(*it)
                        seq[0] += 1
                        pending.append(
                            lambda it=it, pt=pt, off=off:
                            stage2(it[0], it[1], it[2], it[3], pt, off))
                        while len(pending) > DEPTH:
                            pending.popleft()()
                        while pending2 and pending2[0][0] <= seq[0]:
                            pending2.popleft()[1]()
            while pending:
                pending.popleft()()
            while pending2:
                pending2.popleft()[1]()
    nc.compile()
    return nc


def _prep_in_maps(x, w_qkv, b_qkv, w_out):
    xTfull = np.ascontiguousarray(x.reshape(BT, C).T.astype(np.float16))
    in_maps = []
    for core in range(NCORES):
        hs = [core * HPC + i for i in range(HPC)]
        wq = np.ascontiguousarray(np.concatenate(
            [w_qkv[:, sec * C + h * D: sec * C + (h + 1) * D]
             for sec in range(3) for h in hs], axis=1).astype(np.float16))
        bq = np.ascontiguousarray(np.stack(
            [np.concatenate([b_qkv[sec * C + h * D: sec * C + (h + 1) * D] for h in hs])
             for sec in range(3)], axis=1))
        wo = np.ascontiguousarray(np.concatenate(
            [w_out[h * D:(h + 1) * D, :] for h in hs], axis=0).astype(np.float16))
        in_maps.append({"xT": xTfull, "wqkv": wq, "bqkv": bq, "wout": wo})
    return in_maps


LAST_RESULTS = None


def kernel(x, w_qkv, b_qkv, w_out, b_out):
    global LAST_RESULTS
    x = np.asarray(x, np.float32)
    w_qkv = np.asarray(w_qkv, np.float32)
    b_qkv = np.asarray(b_qkv, np.float32)
    w_out = np.asarray(w_out, np.float32)
    b_out = np.asarray(b_out, np.float32)

    if "nc" not in _cache:
        _cache["nc"] = _build_bass()
    nc = _cache["nc"]

    in_maps = _prep_in_maps(x, w_qkv, b_qkv, w_out)
    res = bass_utils.run_bass_kernel_spmd(nc, in_maps, core_ids=list(range(NCORES)))
    LAST_RESULTS = res

    out = np.zeros((BT, C), np.float32)
    for r_ in res.results:
        out += r_["outp"]
    out += b_out
    return out.reshape(B, T, C)


# revision 12
# speedup vs baseline: 1.1581x; 1.1581x over previous
"""Causal self-attention (B=2, T=2048, C=1024, H=16, D=64) on 8 TRN2 NeuronCores.

Tensor-parallel over heads: each core owns 2 heads. w_qkv columns and w_out
rows are sharded by head; x (transposed on host) is replicated. Each core
computes qkv projection -> causal attention -> partial output projection for
its heads; the host sums the 8 partials (the TP all-reduce) and adds b_out.

Device layouts (per core):
  xT      [C=1024, BT=4096]   x transposed (fp16), replicated
  wqkv    [1024, 384]         w_qkv cols  [q_h0 q_h1 k_h0 k_h1 v_h0 v_h1]*64
  bqkv    [128, 3]            matching bias columns per m-tile (fp32)
  wout    [128, 1024]         w_out rows  [h0 h1]*64
  outp    [4096, 1024]        partial output (pre-b_out, fp16)

Matmul operands are fp16 (1 cycle/row, 2-byte weight loads, PSUM accumulates
fp32); scores/softmax stats stay fp32. Scores are computed transposed (keys
on partitions) so softmax needs no transposes: denominators come free from a
ones-column appended to V, and the causal mask is an affine_select on the
exp'd tile. Diagonal score tiles only compute/exp the valid column range
(query >= key); fully masked work is skipped.

The PE sustains only partial utilization once the power throttle engages, so
the schedule keeps the PE queue free of waits: the score->exp->PV chain is
software-pipelined with a global DEPTH-item lag between the score matmul
(stage 1: matmul + exp [+ causal select]) and the PV matmul (stage 2),
interleaving both heads. Per-block normalization and the output projection
(+ fp16 partial store) ride a second queue keyed on stage-1 emission count,
so their PE work lands only after the normalization chain has had time to
complete, and stores overlap attention compute instead of forming a serial
tail.
"""

from collections import deque

import numpy as np

import concourse.bass as bass
from concourse import bacc
import concourse.bass_utils as bass_utils
import concourse.mybir as mybir
from concourse.masks import make_identity
from concourse.tile import TileContext

B, T, C, H, D = 2, 2048, 1024, 16, 64
BT = B * T
NCORES = 8
HPC = H // NCORES          # heads per core
JL = 3 * HPC * D           # 384 local qkv output columns
CL = HPC * D               # 128 local channels into out-proj
KT = 128                   # keys per tile (partition dim of scores^T)
QB = 512                   # queries per block (free dim of scores^T)
NQB = T // QB
NKT = QB // KT             # key tiles per query block (diagonal tiles)
DEPTH = 3                  # stage1 -> stage2 software-pipeline distance
LAG_N = 2                  # stage1 emissions before a block's norm chain
LAG_O = 10                 # stage1 emissions before a block's out-proj
F32 = mybir.dt.float32
F16 = mybir.dt.float16
AF = mybir.ActivationFunctionType

_cache = {}


def _build_bass():
    nc = bacc.Bacc("TRN2", target_bir_lowering=False, debug=False)
    xT = nc.dram_tensor("xT", [C, BT], F16, kind="ExternalInput").ap()
    wqkv = nc.dram_tensor("wqkv", [C, JL], F16, kind="ExternalInput").ap()
    bqkv = nc.dram_tensor("bqkv", [128, 3], F32, kind="ExternalInput").ap()
    wout = nc.dram_tensor("wout", [CL, C], F16, kind="ExternalInput").ap()
    outp = nc.dram_tensor("outp", [BT, C], F16, kind="ExternalOutput").ap()

    with TileContext(nc) as tc:
        with (
            tc.tile_pool(name="const", bufs=1) as const,
            tc.tile_pool(name="xtp", bufs=3) as xtp,
            tc.tile_pool(name="vtp", bufs=3) as vtp,
            tc.tile_pool(name="ptp", bufs=6) as ptp,
            tc.tile_pool(name="stg", bufs=3) as stg,
            tc.tile_pool(name="rbp", bufs=3) as rbp,
            tc.tile_pool(name="obp", bufs=4) as obp,
            tc.tile_pool(name="psS", bufs=3, space="PSUM") as psS,
            tc.tile_pool(name="psT", bufs=1, space="PSUM") as psT,
            tc.tile_pool(name="psPV", bufs=2, space="PSUM") as psPV,
            tc.tile_pool(name="psO", bufs=2, space="PSUM") as psO,
        ):
            # ---- static tensors (DMA order tuned: q-weights then the first
            # x tile, so the PE starts early; wout is only needed ~90us in)
            wm_sb = []
            wm_sb.append(const.tile([128, 8, 128], F16, name="wq_sb"))
            nc.sync.dma_start(
                out=wm_sb[0],
                in_=wqkv[:, 0:128].rearrange("(k p) j -> p k j", p=128))
            xt0 = xtp.tile([128, 8, QB], F16, tag="xt", name="xt")
            nc.sync.dma_start(
                out=xt0, in_=xT[:, 0:QB].rearrange("(k p) t -> p k t", p=128))
            for m in (1, 2):
                wm = const.tile([128, 8, 128], F16, name=f"wm{m}_sb")
                nc.sync.dma_start(
                    out=wm,
                    in_=wqkv[:, m * 128:(m + 1) * 128].rearrange(
                        "(k p) j -> p k j", p=128))
                wm_sb.append(wm)
            bias_sb = const.tile([128, 3], F32)
            nc.sync.dma_start(out=bias_sb, in_=bqkv)
            ident = const.tile([128, 128], F16)
            make_identity(nc, ident)
            qT = const.tile([128, BT], F16)    # rows: [h0 d64 | h1 d64]
            kTt = const.tile([128, BT], F16)
            # V in [t, d] tiles + ones column for softmax denominators
            v_sb = const.tile([128, HPC, B, T // KT, D + 1], F16)
            for h in range(HPC):
                for b_ in range(B):
                    nc.vector.memset(v_sb[:, h, b_, :, D:D + 1], 1.0)
            attnTc = const.tile([128, BT], F16)
            attnT1 = const.tile([64, BT], F16)

            # ---- phase A: qkv projection (qkv^T layout) + V transpose
            for tb in range(BT // QB):
                if tb == 0:
                    xt = xt0
                else:
                    xt = xtp.tile([128, 8, QB], F16, tag="xt", name="xt")
                    nc.sync.dma_start(
                        out=xt,
                        in_=xT[:, tb * QB:(tb + 1) * QB].rearrange(
                            "(k p) t -> p k t", p=128))
                for m in range(3):
                    ps = psS.tile([128, QB], F32, tag="s", name="psp")
                    for k in range(8):
                        nc.tensor.matmul(
                            ps,
                            lhsT=wm_sb[m][:, k, :],
                            rhs=xt[:, k, :],
                            start=(k == 0), stop=(k == 7))
                    if m == 0:
                        nc.scalar.activation(
                            out=qT[:, tb * QB:(tb + 1) * QB], in_=ps,
                            func=AF.Identity, bias=bias_sb[:, 0:1])
                    elif m == 1:
                        nc.scalar.activation(
                            out=kTt[:, tb * QB:(tb + 1) * QB], in_=ps,
                            func=AF.Identity, bias=bias_sb[:, 1:2])
                    else:
                        vt = vtp.tile([128, QB], F16, tag="vt", name="vt")
                        nc.scalar.activation(
                            out=vt, in_=ps, func=AF.Identity, bias=bias_sb[:, 2:3])
                        for c4 in range(QB // 128):
                            t0 = tb * QB + c4 * 128
                            b_, kt = t0 // T, (t0 % T) // KT
                            for h in range(HPC):
                                pst = psT.tile([128, D], F16, tag="pst", name="pst")
                                nc.tensor.transpose(
                                    pst,
                                    vt[h * 64:(h + 1) * 64, c4 * 128:(c4 + 1) * 128],
                                    ident[h * 64:(h + 1) * 64, h * 64:(h + 1) * 64])
                                nc.vector.tensor_copy(
                                    out=v_sb[:, h, b_, kt, 0:D], in_=pst)
            wout_sb = const.tile([128, C], F16)
            nc.sync.dma_start(out=wout_sb, in_=wout)

            # ---- fused phase B+C: attention, normalization, out-proj
            # Stage 1 (emitted immediately): score matmul + exp (+ causal
            # select on diagonal tiles). Stage 2 (DEPTH stage-1 emissions
            # later, via `pending`): the PV matmul, so the PE never waits on
            # the scalar-engine exp. Norm + out-proj closures ride `pending2`
            # keyed on stage-1 emission count.
            seq = [0]
            pending = deque()
            pending2 = deque()   # (due_seq, closure)
            pv_t = {}
            den_t = {}

            def stage1(b_, qb, h, kt, diag):
                hs = slice(h * 64, (h + 1) * 64)
                q0 = b_ * T + qb * QB
                off = (kt - NKT * qb) * KT if diag else 0
                w = QB - off
                ps = psS.tile([128, w], F32, tag="s", name="pss")
                nc.tensor.matmul(
                    ps,
                    lhsT=kTt[hs, b_ * T + kt * KT: b_ * T + (kt + 1) * KT],
                    rhs=qT[hs, q0 + off:q0 + QB],
                    start=True, stop=True)
                pt = ptp.tile([128, w], F16, tag="pt", name="pt")
                nc.scalar.activation(
                    out=pt, in_=ps, func=AF.Exp, scale=float(D) ** -0.5)
                if diag:
                    # keep exp(score) where local query col >= key row
                    nc.gpsimd.affine_select(
                        out=pt, in_=pt,
                        compare_op=mybir.AluOpType.is_ge, fill=0.0,
                        base=0, channel_multiplier=-1, pattern=[[1, w]])
                return pt, off

            def stage2(b_, qb, h, kt, pt, off):
                q0 = b_ * T + qb * QB
                n_kt = (qb + 1) * NKT
                key = (b_, qb, h)
                if key not in pv_t:
                    pv_t[key] = psPV.tile([D + 1, QB], F32, tag="pv", name="pv")
                pv = pv_t[key]
                nc.tensor.matmul(
                    pv[:, off:QB],
                    lhsT=v_sb[:, h, b_, kt, :],
                    rhs=pt,
                    start=(kt == 0), stop=(kt == n_kt - 1))
                if kt == n_kt - 1:
                    dst = (attnTc[0:D, q0:q0 + QB] if h == 0
                           else attnT1[:, q0:q0 + QB])
                    nc.vector.tensor_copy(out=dst, in_=pv[0:D, :])
                    # denominator row: psum p64 -> sbuf p64 -> (dma) den row h
                    dstage = stg.tile([D + 1, QB], F32, tag="dstage", name="dstage")
                    nc.vector.tensor_copy(out=dstage[D:D + 1, :], in_=pv[D:D + 1, :])
                    nc.gpsimd.dma_start(
                        out=den_t[(b_, qb)][h:h + 1, :],
                        in_=dstage[D:D + 1, :])
                    del pv_t[key]
                    if h == HPC - 1:
                        pending2.append(
                            (seq[0] + LAG_N,
                             lambda b_=b_, qb=qb: norm_block(b_, qb)))
                        for j in range(QB // 128):
                            t0 = q0 + j * 128
                            for ch in range(2):
                                pending2.append(
                                    (seq[0] + LAG_O,
                                     lambda t0=t0, ch=ch: outproj(t0, ch)))

            def norm_block(b_, qb):
                dt = den_t.pop((b_, qb))
                recip_t = stg.tile([HPC, QB], F32, tag="recip", name="recip")
                rscr_t = stg.tile([HPC, QB], F32, tag="rscr", name="rscr")
                recip16_t = stg.tile([HPC, QB], F16, tag="recip16", name="recip16")
                nc.vector.reciprocal_approx_accurate(
                    out=recip_t, in_=dt, scratch=rscr_t)
                nc.vector.tensor_copy(out=recip16_t, in_=recip_t)
                for h in range(HPC):
                    r0 = rbp.tile([1, QB], F16, tag="r0", name="r0")
                    nc.gpsimd.dma_start(out=r0, in_=recip16_t[h:h + 1, :])
                    rb = rbp.tile([D, QB], F16, tag="rb", name="rb")
                    nc.gpsimd.partition_broadcast(rb, r0)
                    cols = slice(b_ * T + qb * QB, b_ * T + (qb + 1) * QB)
                    if h == 0:
                        sl = attnTc[0:D, cols]
                        nc.vector.tensor_mul(sl, sl, rb)
                    else:
                        sl = attnT1[:, cols]
                        nc.vector.tensor_mul(sl, sl, rb)
                        nc.gpsimd.dma_start(
                            out=attnTc[D:2 * D, cols], in_=attnT1[:, cols])

            def outproj(t0, ch):
                po = psO.tile([128, QB], F32, tag="po", name="po")
                nc.tensor.matmul(
                    po,
                    lhsT=attnTc[:, t0:t0 + 128],
                    rhs=wout_sb[:, ch * QB:(ch + 1) * QB],
                    start=True, stop=True)
                ob = obp.tile([128, QB], F16, tag="ob", name="ob")
                nc.any.tensor_copy(out=ob, in_=po)
                nc.sync.dma_start(
                    out=outp[t0:t0 + 128, ch * QB:(ch + 1) * QB], in_=ob)

            for b_ in range(B):
                for qb in range(NQB):
                    den_t[(b_, qb)] = stg.tile(
                        [HPC, QB], F32, tag="den", name="den")
                    blk = []
                    for kt in range(NKT * qb):
                        for h in range(HPC):
                            blk.append((b_, qb, h, kt, False))
                    for kt in range(NKT * qb, NKT * (qb + 1)):
                        for h in range(HPC):
                            blk.append((b_, qb, h, kt, True))
                    for it in blk:
                        pt, off = stage1(*it)
                        seq[0] += 1
                        pending.append(
                            lambda it=it, pt=pt, off=off:
                            stage2(it[0], it[1], it[2], it[3], pt, off))
                        while len(pending) > DEPTH:
                            pending.popleft()()
                        while pending2 and pending2[0][0] <= seq[0]:
                            pending2.popleft()[1]()
            while pending:
                pending.popleft()()
            while pending2:
                pending2.popleft()[1]()
    nc.compile()
    return nc


def _prep_in_maps(x, w_qkv, b_qkv, w_out):
    xTfull = np.ascontiguousarray(x.reshape(BT, C).T.astype(np.float16))
    in_maps = []
    for core in range(NCORES):
        hs = [core * HPC + i for i in range(HPC)]
        wq = np.ascontiguousarray(np.concatenate(
            [w_qkv[:, sec * C + h * D: sec * C + (h + 1) * D]
             for sec in range(3) for h in hs], axis=1).astype(np.float16))
        bq = np.ascontiguousarray(np.stack(
            [np.concatenate([b_qkv[sec * C + h * D: sec * C + (h + 1) * D] for h in hs])
             for sec in range(3)], axis=1))
        wo = np.ascontiguousarray(np.concatenate(
            [w_out[h * D:(h + 1) * D, :] for h in hs], axis=0).astype(np.float16))
        in_maps.append({"xT": xTfull, "wqkv": wq, "bqkv": bq, "wout": wo})
    return in_maps


LAST_RESULTS = None


def kernel(x, w_qkv, b_qkv, w_out, b_out):
    global LAST_RESULTS
    x = np.asarray(x, np.float32)
    w_qkv = np.asarray(w_qkv, np.float32)
    b_qkv = np.asarray(b_qkv, np.float32)
    w_out = np.asarray(w_out, np.float32)
    b_out = np.asarray(b_out, np.float32)

    if "nc" not in _cache:
        _cache["nc"] = _build_bass()
    nc = _cache["nc"]

    in_maps = _prep_in_maps(x, w_qkv, b_qkv, w_out)
    res = bass_utils.run_bass_kernel_spmd(nc, in_maps, core_ids=list(range(NCORES)))
    LAST_RESULTS = res

    out = np.zeros((BT, C), np.float32)
    for r_ in res.results:
        out += r_["outp"]
    out += b_out
    return out.reshape(B, T, C)
